# revision 2
# baseline (speedup 1.0000x reference)
"""Trainium2 Bass kernel for AdaptiveGraphLearning (retrieval_knn).

Computes, for X [8192,128], A_raw [8192,8192], lambda scalar:
  Xn = X / max(||X||_2, 1e-12)   (row-normalize)
  S  = Xn @ Xn.T                 (cosine similarity)
  A  = dense top-(K+1) per row with self-edge dropped, row-normalized
  A_final = sigmoid(lam)*A_raw + (1-sigmoid(lam))*A_learned
Returns (A_final, A_learned).

Distribution: row-shard N across 8 cores (1024 rows/core). Each core gets
the full X but ROTATED by its row offset, so in its local coordinates its
rows are 0..1024 and the self-similarity diagonal of row-tile t always
falls at local columns [t*128,(t+1)*128) -- the SPMD graph is identical on
all cores. The host passes X in a [128, 64, 128] partition-major layout
(contiguous DMA); A_raw shards are column-rotated the same way and
downcast to bf16 (halves the dominant input stream; the lam*A_raw term
tolerates the 2^-8 rounding easily at the 2e-2 gate), and the outputs are
un-rotated after the gather.

Top-k without indices: per row, the 11th-largest similarity INCLUDING the
self-edge (which is exactly 1.0 and therefore always rank 1) is the
10-neighbor threshold tau. The DVE max8 scans each 1024-col chunk straight
out of PSUM (no dependency on the ACT drain), giving 64 candidates ->
top-16 via max8 + match_replace + max8; tau = rank 11 = g2[:,2], and the
row sum of the selected 10 is sum(g1) + g2[0..2] - 1.0. Selection is one
fused pass on the SBUF copy of S (diagonal zeroed): SEL = (S >= tau) * S.

Engine split per row-tile: PE does 16 fp32 matmuls into 8 PSUM banks; ACT
drains PSUM->SBUF, casts bf16 A_raw to lam*A_raw (f32), and scales
A_learned; DVE does the PSUM max8 scan, the select, and the final blend.
DMA rings are split: output stores ride the Sync HWDGE ring, A_raw
prefetches ride the Scalar HWDGE ring, so a store waiting on compute can
never head-of-line block a ready prefetch. Epilogue ops and stores are
4096 cols wide (2 MiB DMAs); the last tile uses 2048 to shorten the
drain tail.
"""

import numpy as np

N = 8192
D = 128
NCORES = 8
RPC = N // NCORES   # rows per core
P = 128
TILES = RPC // P    # row tiles per core
MMF = 512           # matmul moving free dim (one PSUM bank, f32)
CH = 1024           # max8 chunk width (two PSUM banks)
NCH = N // CH       # chunks per row: 8
CAND = NCH * 8      # candidates per row: 64
XG = 4              # X prologue groups
XT_PER_G = (N // P) // XG  # x row-tiles per group: 16
EPQ = 4096          # epilogue column chunk (2 MiB stores)
NEP = N // EPQ      # epilogue chunks: 2
LEPQ = 2048         # last-tile epilogue chunk (drain tail)
LNEP = N // LEPQ

LAST_RESULTS = None
_NC_CACHE = None


def _build():
    import concourse.mybir as mybir
    import concourse.tile as tile
    from concourse import bacc
    from concourse.bass import ts
    from concourse.masks import make_identity

    f32 = mybir.dt.float32
    bf16 = mybir.dt.bfloat16
    AF = mybir.ActivationFunctionType
    OP = mybir.AluOpType

    nc = bacc.Bacc("TRN2", target_bir_lowering=False, debug=False,
                   num_devices=NCORES)

    X_d = nc.dram_tensor("X", [P, N], f32, kind="ExternalInput")
    A_d = nc.dram_tensor("A_raw", [RPC, N], bf16, kind="ExternalInput")
    lam_d = nc.dram_tensor("lam", [P, 1], f32, kind="ExternalInput")
    AF_d = nc.dram_tensor("A_final", [RPC, N], f32, kind="ExternalOutput")
    AL_d = nc.dram_tensor("A_learned", [RPC, N], f32, kind="ExternalOutput")

    with tile.TileContext(nc) as tc:
        with (
            tc.tile_pool(name="const", bufs=1) as constp,
            tc.tile_pool(name="xnt", bufs=1) as xntp,
            tc.tile_pool(name="selp", bufs=2) as selp,
            tc.tile_pool(name="arawp", bufs=2) as arawp,
            tc.tile_pool(name="afp", bufs=2) as afp,
            tc.tile_pool(name="small", bufs=2) as smallp,
            tc.tile_pool(name="psum", bufs=4, space="PSUM") as psump,
        ):
            # lambda: sigmoid on device; host replicates the scalar to [128,1]
            lam_sb = constp.tile([P, 1], f32, name="lam_sb")
            nc.sync.dma_start(lam_sb[:], lam_d.ap())
            lam_bc = constp.tile([P, 1], f32, name="lam_bc")
            nc.scalar.activation(lam_bc[:], lam_sb[:], AF.Sigmoid)
            omlam = constp.tile([P, 1], f32, name="omlam")
            nc.scalar.activation(omlam[:], lam_bc[:], AF.Copy, bias=1.0,
                                 scale=-1.0)

            ident = constp.tile([P, P], f32, name="ident")
            make_identity(nc, ident[:])
            # notI: 1 everywhere except 0 on the diagonal
            notI = constp.tile([P, P], f32, name="notI")
            nc.gpsimd.memset(notI[:], 1.0)
            nc.gpsimd.affine_select(
                out=notI[:], in_=notI[:], pattern=[[-1, P]],
                compare_op=OP.not_equal, fill=0.0, base=0,
                channel_multiplier=1)

            araw_tiles = {}
            def fetch_araw(t):
                araw_t = arawp.tile([P, N], bf16, name=f"araw{t}", tag="araw")
                nc.scalar.dma_start(araw_t[:], A_d.ap()[ts(t, P), :])
                araw_tiles[t] = araw_t

            # X prologue: host supplies X pre-transposed as [P, tt, d]
            # (partition-major, contiguous). Row-normalize, PE-transpose
            # into XnT [D, N]; 8 transposes share one PSUM tile so the
            # ACT drains are 1024 wide.
            xnt = xntp.tile([P, N], f32, name="xnt")
            xt = selp.tile([P, N // P, D], f32, name="xt", tag="sel")
            sq = selp.tile([P, N // P, D], f32, name="sq", tag="sel")
            n2 = constp.tile([P, N // P], f32, name="n2")
            invn = constp.tile([P, N // P], f32, name="invn")
            xr = X_d.ap().rearrange("p (t d) -> p t d", d=D)
            for g in range(XG):
                gsl = ts(g, XT_PER_G)
                nc.sync.dma_start(xt[:, gsl, :], xr[:, gsl, :])
                nc.scalar.activation(sq[:, gsl, :], xt[:, gsl, :], AF.Square)
                nc.vector.reduce_sum(n2[:, gsl], sq[:, gsl, :],
                                     axis=mybir.AxisListType.X)
                nc.scalar.activation(invn[:, gsl], n2[:, gsl], AF.Sqrt)
                # A_raw prefetch triggers sit in the ACT stream after the
                # squares so the X DMAs keep the SDMA engines to themselves
                # while they are the critical path.
                if g == 1:
                    fetch_araw(0)
                if g == 3:
                    fetch_araw(1)
                nc.vector.tensor_scalar_max(invn[:, gsl], invn[:, gsl], 1e-12)
                nc.vector.reciprocal(invn[:, gsl], invn[:, gsl])
                nc.vector.tensor_mul(
                    xt[:, gsl, :], xt[:, gsl, :],
                    invn[:, gsl, None].to_broadcast((P, XT_PER_G, D)))
                for half in range(2):
                    base = g * XT_PER_G + half * 8
                    pt = psump.tile([P, CH], f32, name=f"tp{base}", tag="mm")
                    for k in range(8):
                        nc.tensor.transpose(pt[:, ts(k, P)],
                                            xt[:, base + k, :], ident[:])
                    nc.scalar.copy(xnt[:, base * P:(base + 8) * P], pt[:])

            def ar_chunk(t, q, w, nq):
                # af <- lam * A_raw (bf16 -> f32 cast on ACT)
                af_t = af_tiles[t]
                nc.scalar.activation(af_t[:, ts(q, w)],
                                     araw_tiles[t][:, ts(q, w)], AF.Copy,
                                     scale=lam_bc[:])
                if q == nq - 1:
                    del araw_tiles[t]
                    if t + 2 < TILES:
                        fetch_araw(t + 2)

            af_tiles = {}
            def alloc_af(t):
                af_tiles[t] = afp.tile([P, N], f32, name=f"af{t}", tag="af")

            # pending: deferred epilogue/cast work popped one item per
            # matmul chunk of the NEXT tile, so ACT/DVE/Sync streams stay
            # interleaved with the copy/scan stream.
            alloc_af(0)
            pending = [lambda q=q: ar_chunk(0, q, EPQ, NEP) for q in range(NEP)]

            for t in range(TILES):
                s_t = selp.tile([P, N], f32, name=f"s{t}", tag="sel")
                cand = smallp.tile([P, CAND], f32, name=f"cand{t}", tag="cand")
                for c in range(NCH):
                    pm = psump.tile([P, CH], f32, name=f"mm{t}_{c}", tag="mm")
                    nc.tensor.matmul(pm[:, 0:MMF], xnt[:, ts(t, P)],
                                     xnt[:, ts(2 * c, MMF)],
                                     start=True, stop=True)
                    nc.tensor.matmul(pm[:, MMF:CH], xnt[:, ts(t, P)],
                                     xnt[:, ts(2 * c + 1, MMF)],
                                     start=True, stop=True)
                    nc.scalar.copy(s_t[:, ts(c, CH)], pm[:])
                    # scan straight from PSUM: includes the self-edge (==1.0,
                    # always the row max), so thresholds shift one rank
                    nc.vector.max(cand[:, ts(c, 8)], pm[:])
                    if c == 0:
                        # zero the self-similarity diagonal in the SBUF copy
                        nc.vector.tensor_mul(s_t[:, ts(t, P)],
                                             s_t[:, ts(t, P)], notI[:])
                    if pending:
                        pending.pop(0)()
                while pending:
                    pending.pop(0)()

                g1 = smallp.tile([P, 8], f32, name=f"g1_{t}", tag="g1")
                nc.vector.max(g1[:], cand[:])
                nc.vector.match_replace(out=cand[:], in_to_replace=g1[:],
                                        in_values=cand[:], imm_value=-1e30)
                g2 = smallp.tile([P, 8], f32, name=f"g2_{t}", tag="g2")
                nc.vector.max(g2[:], cand[:])

                # selected-10 rowsum = sum(g1) + g2[0]+g2[1]+g2[2] - self(1.0)
                rs1 = smallp.tile([P, 1], f32, name=f"rs1_{t}", tag="rs1")
                nc.vector.reduce_sum(rs1[:], g1[:], axis=mybir.AxisListType.X)
                rs2 = smallp.tile([P, 1], f32, name=f"rs2_{t}", tag="rs2")
                nc.vector.reduce_sum(rs2[:], g2[:, 0:3],
                                     axis=mybir.AxisListType.X)
                den = smallp.tile([P, 1], f32, name=f"den{t}", tag="den")
                nc.vector.tensor_add(den[:], rs1[:], rs2[:])
                nc.vector.tensor_scalar_add(den[:], den[:], 1e-6 - 1.0)
                invr = smallp.tile([P, 1], f32, name=f"invr{t}", tag="invr")
                nc.vector.reciprocal(invr[:], den[:])

                # SEL = (S >= tau) * S, in place on s_t; tau = g2[:,2]
                # (rank 11 including the self-edge). Two halves so the
                # first AL chunk can start as soon as its half is done.
                nc.vector.scalar_tensor_tensor(
                    out=s_t[:, 0:N // 2], in0=s_t[:, 0:N // 2],
                    scalar=g2[:, 2:3], in1=s_t[:, 0:N // 2],
                    op0=OP.is_ge, op1=OP.mult)
                nc.vector.scalar_tensor_tensor(
                    out=s_t[:, N // 2:N], in0=s_t[:, N // 2:N],
                    scalar=g2[:, 2:3], in1=s_t[:, N // 2:N],
                    op0=OP.is_ge, op1=OP.mult)

                w, nq = (LEPQ, LNEP) if t == TILES - 1 else (EPQ, NEP)

                def al_chunk(t=t, s_t=s_t, invr=invr, q=0, w=w):
                    qs = ts(q, w)
                    nc.scalar.activation(s_t[:, qs], s_t[:, qs], AF.Copy,
                                         scale=invr[:])
                    nc.sync.dma_start(AL_d.ap()[ts(t, P), qs], s_t[:, qs])

                def af_chunk(t=t, s_t=s_t, q=0, w=w):
                    # A_final = omlam*A_learned + lam*A_raw, in place on af
                    qs = ts(q, w)
                    af_t = af_tiles[t]
                    nc.vector.scalar_tensor_tensor(
                        out=af_t[:, qs], in0=s_t[:, qs], scalar=omlam[:],
                        in1=af_t[:, qs], op0=OP.mult, op1=OP.add)
                    nc.sync.dma_start(AF_d.ap()[ts(t, P), qs], af_t[:, qs])

                pending = []
                for q in range(nq):
                    pending.append(lambda q=q: al_chunk(q=q))
                    pending.append(lambda q=q: af_chunk(q=q))
                if t + 1 < TILES:
                    alloc_af(t + 1)
                    pending += [lambda q=q, tt=t + 1: ar_chunk(tt, q, EPQ, NEP)
                                for q in range(NEP)]

            while pending:
                pending.pop(0)()

    nc.compile()
    return nc


def kernel(X, A_raw, lambda_param):
    global LAST_RESULTS, _NC_CACHE
    import ml_dtypes
    from concourse.bass_utils import run_bass_kernel_spmd

    X = np.asarray(X, dtype=np.float32)
    A_raw = np.asarray(A_raw, dtype=np.float32)
    lam = float(np.asarray(lambda_param, dtype=np.float32).reshape(()))

    if _NC_CACHE is None:
        _NC_CACHE = _build()
    nc = _NC_CACHE

    lam_in = np.full((P, 1), lam, dtype=np.float32)
    in_maps = []
    for c in range(NCORES):
        r0 = c * RPC
        Xrot = np.roll(X, -r0, axis=0)
        # [P, N] partition-major: Xp[p, tt*D + d] = Xrot[tt*P + p, d]
        Xp = np.ascontiguousarray(
            Xrot.reshape(N // P, P, D).transpose(1, 0, 2).reshape(P, N))
        Arot = np.roll(A_raw[r0:r0 + RPC], -r0, axis=1)
        in_maps.append({
            "X": Xp,
            "A_raw": np.ascontiguousarray(Arot.astype(ml_dtypes.bfloat16)),
            "lam": lam_in,
        })

    res = run_bass_kernel_spmd(nc, in_maps, core_ids=list(range(NCORES)))
    LAST_RESULTS = res

    A_final = np.empty((N, N), dtype=np.float32)
    A_learned = np.empty((N, N), dtype=np.float32)
    for c in range(NCORES):
        r0 = c * RPC
        A_final[r0:r0 + RPC] = np.roll(res.results[c]["A_final"], r0, axis=1)
        A_learned[r0:r0 + RPC] = np.roll(res.results[c]["A_learned"], r0,
                                         axis=1)
    return A_final, A_learned


# revision 3
# speedup vs baseline: 1.1869x; 1.1869x over previous
"""Trainium2 Bass kernel for AdaptiveGraphLearning (retrieval_knn).

Computes, for X [8192,128], A_raw [8192,8192], lambda scalar:
  Xn = X / max(||X||_2, 1e-12)   (row-normalize)
  S  = Xn @ Xn.T                 (cosine similarity)
  A  = dense top-(K+1) per row with self-edge dropped, row-normalized
  A_final = sigmoid(lam)*A_raw + (1-sigmoid(lam))*A_learned
Returns (A_final, A_learned).

Distribution: row-shard N across 8 cores (1024 rows/core). Each core gets
the full X but ROTATED by its row offset, so in its local coordinates its
rows are 0..1024 and the self-similarity diagonal of row-tile t always
falls at local columns [t*128,(t+1)*128) -- the SPMD graph is identical on
all cores. The host passes X in a [128, 64, 128] partition-major layout
(contiguous DMA); A_raw shards are column-rotated the same way and
downcast to bf16 (halves the dominant input stream; the lam*A_raw term
tolerates the 2^-8 rounding easily at the 2e-2 gate), and the outputs are
un-rotated after the gather.

Top-k without indices: per row, the 11th-largest similarity INCLUDING the
self-edge (which is exactly 1.0 and therefore always rank 1) is the
10-neighbor threshold tau: per-1024-chunk max8 -> 64 candidates -> top-16
via max8 + match_replace + max8; tau = rank 11 = g2[:,2]; the row sum of
the selected 10 is sum(g1) + g2[0..2] - 1.0.

Pipeline: engine queues execute in emission order, so each tile window is
emitted in a dependency-safe order. DVE runs the PREVIOUS tile's blends
first (they only need sel(t-1), finished at the end of the last window),
then this tile's chunk scans, candidates, and the two select halves. ACT
runs psum->sbuf copies c0-c3 (which gate matmuls c4-c7 through PSUM
reuse), then the previous tile's A_learned scales, then copies c4-c7,
then the lam*A_raw casts for this tile. The blend reads SEL directly
(scalar = omlam*invr) so it does NOT wait on the A_learned scale; the
in-place scale gets a WAR dep on the blend's read instead. Stores issue
on the Sync HWDGE ring in readiness order (AF before AL); A_raw
prefetches ride the Scalar HWDGE ring so they never queue behind a
store that is still waiting on compute.
"""

import numpy as np

N = 8192
D = 128
NCORES = 8
RPC = N // NCORES   # rows per core
P = 128
TILES = RPC // P    # row tiles per core
MMF = 512           # matmul moving free dim (one PSUM bank, f32)
CH = 1024           # max8 chunk width (two PSUM banks)
NCH = N // CH       # chunks per row: 8
CAND = NCH * 8      # candidates per row: 64
XG = 4              # X prologue groups
XT_PER_G = (N // P) // XG  # x row-tiles per group: 16
EPQ = 4096          # epilogue column chunk (2 MiB stores)
NEP = N // EPQ      # epilogue chunks: 2
LEPQ = 2048         # last-tile epilogue chunk (drain tail)
LNEP = N // LEPQ

LAST_RESULTS = None
_NC_CACHE = None


def _build():
    import concourse.mybir as mybir
    import concourse.tile as tile
    from concourse import bacc
    from concourse.bass import ts
    from concourse.masks import make_identity

    f32 = mybir.dt.float32
    bf16 = mybir.dt.bfloat16
    AF = mybir.ActivationFunctionType
    OP = mybir.AluOpType

    nc = bacc.Bacc("TRN2", target_bir_lowering=False, debug=False,
                   num_devices=NCORES)

    X_d = nc.dram_tensor("X", [P, N], f32, kind="ExternalInput")
    A_d = nc.dram_tensor("A_raw", [RPC, N], bf16, kind="ExternalInput")
    lam_d = nc.dram_tensor("lam", [P, 1], f32, kind="ExternalInput")
    AF_d = nc.dram_tensor("A_final", [RPC, N], f32, kind="ExternalOutput")
    AL_d = nc.dram_tensor("A_learned", [RPC, N], f32, kind="ExternalOutput")

    with tile.TileContext(nc) as tc:
        with (
            tc.tile_pool(name="const", bufs=1) as constp,
            tc.tile_pool(name="xnt", bufs=1) as xntp,
            tc.tile_pool(name="selp", bufs=2) as selp,
            tc.tile_pool(name="arawp", bufs=2) as arawp,
            tc.tile_pool(name="afp", bufs=2) as afp,
            tc.tile_pool(name="small", bufs=2) as smallp,
            tc.tile_pool(name="psum", bufs=4, space="PSUM") as psump,
        ):
            # lambda: sigmoid on device; host replicates the scalar to [128,1]
            lam_sb = constp.tile([P, 1], f32, name="lam_sb")
            nc.sync.dma_start(lam_sb[:], lam_d.ap())
            lam_bc = constp.tile([P, 1], f32, name="lam_bc")
            nc.scalar.activation(lam_bc[:], lam_sb[:], AF.Sigmoid)
            omlam = constp.tile([P, 1], f32, name="omlam")
            nc.scalar.activation(omlam[:], lam_bc[:], AF.Copy, bias=1.0,
                                 scale=-1.0)

            ident = constp.tile([P, P], f32, name="ident")
            make_identity(nc, ident[:])
            # notI: 1 everywhere except 0 on the diagonal
            notI = constp.tile([P, P], f32, name="notI")
            nc.gpsimd.memset(notI[:], 1.0)
            nc.gpsimd.affine_select(
                out=notI[:], in_=notI[:], pattern=[[-1, P]],
                compare_op=OP.not_equal, fill=0.0, base=0,
                channel_multiplier=1)

            araw_tiles = {}
            def fetch_araw(t):
                araw_t = arawp.tile([P, N], bf16, name=f"araw{t}", tag="araw")
                nc.scalar.dma_start(araw_t[:], A_d.ap()[ts(t, P), :])
                araw_tiles[t] = araw_t

            # X prologue: host supplies X pre-transposed as [P, tt, d]
            # (partition-major, contiguous). Row-normalize, PE-transpose
            # into XnT [D, N]; 8 transposes share one PSUM tile so the
            # ACT drains are 1024 wide.
            xnt = xntp.tile([P, N], f32, name="xnt")
            xt = selp.tile([P, N // P, D], f32, name="xt", tag="sel")
            sq = selp.tile([P, N // P, D], f32, name="sq", tag="sel")
            n2 = constp.tile([P, N // P], f32, name="n2")
            invn = constp.tile([P, N // P], f32, name="invn")
            xr = X_d.ap().rearrange("p (t d) -> p t d", d=D)
            for g in range(XG):
                gsl = ts(g, XT_PER_G)
                nc.sync.dma_start(xt[:, gsl, :], xr[:, gsl, :])
                nc.scalar.activation(sq[:, gsl, :], xt[:, gsl, :], AF.Square)
                nc.vector.reduce_sum(n2[:, gsl], sq[:, gsl, :],
                                     axis=mybir.AxisListType.X)
                nc.scalar.activation(invn[:, gsl], n2[:, gsl], AF.Sqrt)
                # A_raw prefetch triggers sit in the ACT stream after the
                # squares so the X DMAs keep the SDMA engines to themselves
                # while they are the critical path.
                if g == 1:
                    fetch_araw(0)
                if g == 3:
                    fetch_araw(1)
                nc.vector.tensor_scalar_max(invn[:, gsl], invn[:, gsl], 1e-12)
                nc.vector.reciprocal(invn[:, gsl], invn[:, gsl])
                nc.vector.tensor_mul(
                    xt[:, gsl, :], xt[:, gsl, :],
                    invn[:, gsl, None].to_broadcast((P, XT_PER_G, D)))
                for half in range(2):
                    base = g * XT_PER_G + half * 8
                    pt = psump.tile([P, CH], f32, name=f"tp{base}", tag="mm")
                    for k in range(8):
                        nc.tensor.transpose(pt[:, ts(k, P)],
                                            xt[:, base + k, :], ident[:])
                    nc.scalar.copy(xnt[:, base * P:(base + 8) * P], pt[:])

            af_tiles = {}
            prev = None  # (t, s_t, invr, w2, w, nq) of the previous tile

            for t in range(TILES):
                s_t = selp.tile([P, N], f32, name=f"s{t}", tag="sel")
                cand = smallp.tile([P, CAND], f32, name=f"cand{t}", tag="cand")
                af_tiles[t] = afp.tile([P, N], f32, name=f"af{t}", tag="af")

                # previous tile's blends first: they only need sel(t-1) and
                # af(t-1), both ready, so DVE starts the window immediately
                # and the AF stores hit the ring early.
                if prev is not None:
                    pt_, ps, _, pw2, pw, pnq = prev
                    for q in range(pnq):
                        qs = ts(q, pw)
                        nc.vector.scalar_tensor_tensor(
                            out=af_tiles[pt_][:, qs], in0=ps[:, qs],
                            scalar=pw2[:], in1=af_tiles[pt_][:, qs],
                            op0=OP.mult, op1=OP.add)
                        nc.sync.dma_start(AF_d.ap()[ts(pt_, P), qs],
                                          af_tiles[pt_][:, qs])

                # chunks c0-c3: matmul -> ACT drain -> DVE max8 scan (SBUF)
                for c in range(NCH):
                    if c == NCH // 2 and prev is not None:
                        # A_learned scales of t-1 (in-place on s_(t-1), WAR
                        # on the blend reads above) + stores
                        pt_, ps, pinvr, _, pw, pnq = prev
                        for q in range(pnq):
                            qs = ts(q, pw)
                            nc.scalar.activation(ps[:, qs], ps[:, qs],
                                                 AF.Copy, scale=pinvr[:])
                            nc.sync.dma_start(AL_d.ap()[ts(pt_, P), qs],
                                              ps[:, qs])
                    pm = psump.tile([P, CH], f32, name=f"mm{t}_{c}", tag="mm")
                    nc.tensor.matmul(pm[:, 0:MMF], xnt[:, ts(t, P)],
                                     xnt[:, ts(2 * c, MMF)],
                                     start=True, stop=True)
                    nc.tensor.matmul(pm[:, MMF:CH], xnt[:, ts(t, P)],
                                     xnt[:, ts(2 * c + 1, MMF)],
                                     start=True, stop=True)
                    nc.scalar.copy(s_t[:, ts(c, CH)], pm[:])
                    nc.vector.max(cand[:, ts(c, 8)], s_t[:, ts(c, CH)])
                    if c == 0:
                        # zero the self-similarity diagonal AFTER the scan:
                        # the self-edge (==1.0) is wanted in the candidates
                        nc.vector.tensor_mul(s_t[:, ts(t, P)],
                                             s_t[:, ts(t, P)], notI[:])

                # lam*A_raw casts for this tile (consumed by blends in the
                # next window); freeing araw_t afterwards lets the t+2
                # prefetch start on the Scalar ring.
                for q in range(NEP):
                    nc.scalar.activation(af_tiles[t][:, ts(q, EPQ)],
                                         araw_tiles[t][:, ts(q, EPQ)],
                                         AF.Copy, scale=lam_bc[:])
                del araw_tiles[t]
                if t + 2 < TILES:
                    fetch_araw(t + 2)

                g1 = smallp.tile([P, 8], f32, name=f"g1_{t}", tag="g1")
                nc.vector.max(g1[:], cand[:])
                nc.vector.match_replace(out=cand[:], in_to_replace=g1[:],
                                        in_values=cand[:], imm_value=-1e30)
                g2 = smallp.tile([P, 8], f32, name=f"g2_{t}", tag="g2")
                nc.vector.max(g2[:], cand[:])

                # selected-10 rowsum = sum(g1) + g2[0]+g2[1]+g2[2] - self(1.0)
                rs1 = smallp.tile([P, 1], f32, name=f"rs1_{t}", tag="rs1")
                nc.vector.reduce_sum(rs1[:], g1[:], axis=mybir.AxisListType.X)
                rs2 = smallp.tile([P, 1], f32, name=f"rs2_{t}", tag="rs2")
                nc.vector.reduce_sum(rs2[:], g2[:, 0:3],
                                     axis=mybir.AxisListType.X)
                den = smallp.tile([P, 1], f32, name=f"den{t}", tag="den")
                nc.vector.tensor_add(den[:], rs1[:], rs2[:])
                nc.vector.tensor_scalar_add(den[:], den[:], 1e-6 - 1.0)
                invr = smallp.tile([P, 1], f32, name=f"invr{t}", tag="invr")
                nc.vector.reciprocal(invr[:], den[:])
                # blend scalar: omlam/row_sum, so the blend reads SEL directly
                w2 = smallp.tile([P, 1], f32, name=f"w2_{t}", tag="w2")
                nc.vector.tensor_mul(w2[:], invr[:], omlam[:])

                # SEL = (S >= tau) * S, in place on s_t; tau = g2[:,2]
                # (rank 11 including the self-edge). Two halves so the
                # first epilogue chunk can start as soon as its half is done.
                for h in range(2):
                    hs = ts(h, N // 2)
                    nc.vector.scalar_tensor_tensor(
                        out=s_t[:, hs], in0=s_t[:, hs], scalar=g2[:, 2:3],
                        in1=s_t[:, hs], op0=OP.is_ge, op1=OP.mult)

                w, nq = (LEPQ, LNEP) if t == TILES - 1 else (EPQ, NEP)
                prev = (t, s_t, invr, w2, w, nq)

            # drain: epilogue of the last tile
            pt_, ps, pinvr, pw2, pw, pnq = prev
            for q in range(pnq):
                qs = ts(q, pw)
                nc.vector.scalar_tensor_tensor(
                    out=af_tiles[pt_][:, qs], in0=ps[:, qs], scalar=pw2[:],
                    in1=af_tiles[pt_][:, qs], op0=OP.mult, op1=OP.add)
                nc.sync.dma_start(AF_d.ap()[ts(pt_, P), qs],
                                  af_tiles[pt_][:, qs])
                nc.scalar.activation(ps[:, qs], ps[:, qs], AF.Copy,
                                     scale=pinvr[:])
                nc.sync.dma_start(AL_d.ap()[ts(pt_, P), qs], ps[:, qs])

    nc.compile()
    return nc


def kernel(X, A_raw, lambda_param):
    global LAST_RESULTS, _NC_CACHE
    import ml_dtypes
    from concourse.bass_utils import run_bass_kernel_spmd

    X = np.asarray(X, dtype=np.float32)
    A_raw = np.asarray(A_raw, dtype=np.float32)
    lam = float(np.asarray(lambda_param, dtype=np.float32).reshape(()))

    if _NC_CACHE is None:
        _NC_CACHE = _build()
    nc = _NC_CACHE

    lam_in = np.full((P, 1), lam, dtype=np.float32)
    in_maps = []
    for c in range(NCORES):
        r0 = c * RPC
        Xrot = np.roll(X, -r0, axis=0)
        # [P, N] partition-major: Xp[p, tt*D + d] = Xrot[tt*P + p, d]
        Xp = np.ascontiguousarray(
            Xrot.reshape(N // P, P, D).transpose(1, 0, 2).reshape(P, N))
        Arot = np.roll(A_raw[r0:r0 + RPC], -r0, axis=1)
        in_maps.append({
            "X": Xp,
            "A_raw": np.ascontiguousarray(Arot.astype(ml_dtypes.bfloat16)),
            "lam": lam_in,
        })

    res = run_bass_kernel_spmd(nc, in_maps, core_ids=list(range(NCORES)))
    LAST_RESULTS = res

    A_final = np.empty((N, N), dtype=np.float32)
    A_learned = np.empty((N, N), dtype=np.float32)
    for c in range(NCORES):
        r0 = c * RPC
        A_final[r0:r0 + RPC] = np.roll(res.results[c]["A_final"], r0, axis=1)
        A_learned[r0:r0 + RPC] = np.roll(res.results[c]["A_learned"], r0,
                                         axis=1)
    return A_final, A_learned
